# revision 16
# baseline (speedup 1.0000x reference)
"""ChildSum tree RNN over a batch of complete binary trees — Trainium2 Bass kernel.

Strategy (data-parallel over trees, 8 cores x 128 trees):
  - States are kept *transposed* in SBUF ([feature, tree-node-column]) so every
    level's matmul chains directly into the next with no on-device transposes.
  - The per-node op select is algebraic: with premasked operand pairs
    (s_and, s_or) = (h*(1-op), h*op), a single PSUM accumulation
    W_and@s_and + W_or@s_or computes the selected transform. The select for
    level 0 is baked into the inputs on the host (fp16 premasked leaves);
    levels 1-6 derive s_or = ss*bm (bm = 0.5*op, exact in fp16) and
    s_and = 0.5*ss - s_or via one scalar_tensor_tensor.
  - Block-local "deal" column order: within each block the two siblings of a
    pair sit at offsets (p, p+HB), so the sibling-mean is a tensor_add of two
    contiguous fp16 runs (DVE 2x mode). The next level's matmul un-deals via a
    stride-2 rhs access pattern (free for the PE).
  - DMA instruction count is kept minimal (fixed ~0.6us sequencer cost per
    dma_start): one bm load per level, 2048-column input loads; mask/weight
    DMAs issue from the idle GPSIMD sequencer.
  - The sibling sums are left unscaled; the mean's x0.5 rides in the next
    level's masks, and the root's x0.5 is applied on the host.
"""

import sys

for _p in ("/opt/trn_rl_repo",):
    if _p not in sys.path:
        sys.path.insert(0, _p)

import numpy as np

import concourse.bacc as bacc
import concourse.mybir as mybir
import concourse.tile as tile
from concourse import bass_utils

N_CORES = 8
B, L, M = 1024, 128, 256
BC = B // N_CORES          # trees per core
R0 = BC * L                # level-0 child columns per core (16384)
DEPTH = 7
LEVEL_R = [R0 >> l for l in range(DEPTH)]      # child columns per level
LEVEL_N = [64 >> l for l in range(DEPTH)]      # parents per tree per level
LEVEL_OFF = [0, 64, 96, 112, 120, 124, 126]    # offsets into ops[:, :]
BLK = 1024                                     # child cols per block
XSB = 2048                                     # level-0 x cols per DMA
MSK_SEG = [R0 >> (l + 1) for l in range(DEPTH - 1)]   # bm segment sizes
MSK_TOT = sum(MSK_SEG)                         # 16128

F16 = mybir.dt.float16


def _body(nc, xa, xo, wa5, wo5, mb, outT, tc):
    f32 = mybir.dt.float32
    Alu = mybir.AluOpType

    with (
        tc.tile_pool(name="wpool", bufs=1) as wpool,
        tc.tile_pool(name="spool", bufs=1) as spool,
        tc.tile_pool(name="xpool", bufs=2) as xpool,
        tc.tile_pool(name="vpool", bufs=3) as vpool,
        tc.tile_pool(name="mpool", bufs=1) as mpool,
        tc.tile_pool(name="ppool", bufs=2, space="PSUM") as ppool,
    ):
        # Stationary weights [contract-chunk m (128 part), out-feature k (256)]
        wt = {}
        for nm, dram in (("wa5", wa5), ("wo5", wo5)):
            for C in range(2):
                t = wpool.tile([128, 256], F16, name=f"{nm}{C}", tag=f"{nm}{C}")
                nc.gpsimd.dma_start(out=t, in_=dram[C * 128:(C + 1) * 128, :])
                wt[(nm, C)] = t

        def epilogue(ss3, bm_t, s_and, s_or, j, HB):
            """premask: s_or = ss*bm, s_and = 0.5*ss - s_or (exact in fp16)"""
            sa3 = s_and.rearrange("p (h q) -> p h q", h=2)
            so3 = s_or.rearrange("p (h q) -> p h q", h=2)
            bm3 = (bm_t[:, j * HB:(j + 1) * HB]
                   .unsqueeze(1).broadcast_to([128, 2, HB]))
            osl = slice(j * HB, (j + 1) * HB)
            nc.vector.tensor_mul(so3[:, :, osl], ss3, bm3)
            nc.vector.scalar_tensor_tensor(
                sa3[:, :, osl], ss3, 0.5, so3[:, :, osl],
                Alu.mult, Alu.subtract)

        moff = 0
        sa_prev = so_prev = None
        for lvl in range(DEPTH):
            R = LEVEL_R[lvl]
            W = min(BLK, R)
            HB = W // 2
            last = (lvl == DEPTH - 1)
            etag = "e" if lvl % 2 == 0 else "o"
            if not last:
                s_and = spool.tile([128, R], F16, tag=f"sa_{etag}",
                                   name=f"sand{lvl}")
                s_or = spool.tile([128, R], F16, tag=f"so_{etag}",
                                  name=f"sor{lvl}")
                bm_t = mpool.tile([128, MSK_SEG[0]], F16, tag="bm",
                                  name=f"bm{lvl}")
                nc.gpsimd.dma_start(
                    out=bm_t[:, 0:MSK_SEG[lvl]],
                    in_=mb[:, moff:moff + MSK_SEG[lvl]])
            else:
                s_fin = spool.tile([128, 2 * BC], f32, tag="sfin",
                                   name="sfin")

            if lvl > 0:
                # rhs views: h-half C, sibling t, stride-2 over pairs
                sa_v = sa_prev.rearrange("p (h g t) -> p h t g", h=2, t=2)
                so_v = so_prev.rearrange("p (h g t) -> p h t g", h=2, t=2)

            for j in range(R // W):
                # --- matmuls: single accumulated PSUM tensor -------------
                T = ppool.tile([128, 2 * W], f32, tag="tsel",
                               name=f"T{lvl}_{j}")
                if lvl == 0:
                    if j % (XSB // W) == 0:
                        xts = {}
                        for nm_t, dram in (("xa", xa), ("xo", xo)):
                            for C in range(2):
                                xt = xpool.tile(
                                    [128, XSB], F16, tag=f"{nm_t}{C}",
                                    name=f"{nm_t}{C}_{j}")
                                base = (j // (XSB // W)) * XSB
                                nc.sync.dma_start(
                                    out=xt,
                                    in_=dram[C * 128:(C + 1) * 128,
                                             base:base + XSB])
                                xts[(nm_t, C)] = xt
                    ib = j % (XSB // W)
                    for Mo in range(2):
                        for n in range(W // 512):
                            out_ap = T[:, Mo * W + n * 512:
                                       Mo * W + n * 512 + 512]
                            lo = ib * W + n * 512
                            first = True
                            for nm_t, wnm in (("xa", "wa5"), ("xo", "wo5")):
                                for C in range(2):
                                    nc.tensor.matmul(
                                        out_ap,
                                        wt[(wnm, C)][:, Mo * 128:
                                                     (Mo + 1) * 128],
                                        xts[(nm_t, C)][:, lo:lo + 512],
                                        start=first,
                                        stop=(nm_t == "xo" and C == 1))
                                    first = False
                else:
                    NS = min(HB, 512)
                    for Mo in range(2):
                        for t in range(2):
                            g0 = j * HB
                            out_ap = T[:, Mo * W + t * HB:
                                       Mo * W + t * HB + NS]
                            first = True
                            for sv, wnm in ((sa_v, "wa5"), (so_v, "wo5")):
                                for C in range(2):
                                    nc.tensor.matmul(
                                        out_ap,
                                        wt[(wnm, C)][:, Mo * 128:
                                                     (Mo + 1) * 128],
                                        sv[:, C, t, g0:g0 + NS],
                                        start=first,
                                        stop=(wnm == "wo5" and C == 1))
                                    first = False

                # --- tanh + sibling sum + premask ------------------------
                v = vpool.tile([128, 2 * BLK], F16, tag="v",
                               name=f"v{lvl}_{j}")
                nc.scalar.activation(v[:, 0:2 * W], T,
                                     mybir.ActivationFunctionType.Tanh)

                v4 = v[:, 0:2 * W].rearrange("p (h t q) -> p h t q",
                                             h=2, t=2)
                if not last:
                    ss = vpool.tile([128, BLK], F16, tag="ss",
                                    name=f"ss{lvl}_{j}")
                    ss3 = ss[:, 0:2 * HB].rearrange("p (h q) -> p h q", h=2)
                    nc.vector.tensor_add(ss3, v4[:, :, 0, :], v4[:, :, 1, :])
                    epilogue(ss3, bm_t, s_and, s_or, j, HB)
                else:
                    # root: fp32 sum, x0.5 applied on host
                    s3 = s_fin.rearrange("p (h q) -> p h q", h=2)
                    nc.vector.tensor_add(s3, v4[:, :, 0, :], v4[:, :, 1, :])

            if not last:
                moff += MSK_SEG[lvl]
                sa_prev, so_prev = s_and, s_or

        nc.sync.dma_start(out=outT, in_=s_fin)


_NC_CACHE = {}


def _get_nc(reps=1):
    key = ("nc", reps)
    if key not in _NC_CACHE:
        f32 = mybir.dt.float32
        nc = bacc.Bacc("TRN2", target_bir_lowering=False, debug=False)
        xa = nc.dram_tensor("xa", [M, R0], F16, kind="ExternalInput").ap()
        xo = nc.dram_tensor("xo", [M, R0], F16, kind="ExternalInput").ap()
        wa5 = nc.dram_tensor("wa5", [M, M], F16, kind="ExternalInput").ap()
        wo5 = nc.dram_tensor("wo5", [M, M], F16, kind="ExternalInput").ap()
        mb = nc.dram_tensor("mb", [128, MSK_TOT], F16,
                            kind="ExternalInput").ap()
        outT = nc.dram_tensor("outT", [128, 2 * BC], f32,
                              kind="ExternalOutput").ap()
        with tile.TileContext(nc) as tc:
            for _ in range(reps):
                _body(nc, xa, xo, wa5, wo5, mb, outT, tc)
        nc.compile()
        _NC_CACHE[key] = nc
    return _NC_CACHE[key]


def _deal_index():
    """pos -> flat leaf index (b*L + leaf) for the level-0 column order.
    Block-local deal over 1024-col blocks: siblings at (p, p+512)."""
    p = np.arange(R0)
    blk = p >> 10
    t = (p >> 9) & 1
    loc = p & 511
    g = blk * 512 + loc
    b = g >> 6
    i = g & 63
    return b * L + 2 * i + t, b, i


_DEAL = _deal_index()


def make_in_maps(inputs, ops, W_and, W_or):
    f16 = np.float16
    x = np.asarray(inputs, dtype=np.float32)
    opsA = np.asarray(ops)
    waT = np.asarray(W_and, dtype=np.float32).T
    woT = np.asarray(W_or, dtype=np.float32).T
    wa5 = np.ascontiguousarray(waT.astype(f16))
    wo5 = np.ascontiguousarray(woT.astype(f16))
    leaf_idx, db, di = _DEAL
    in_maps = []
    for c in range(N_CORES):
        xc_flat = x[c * BC:(c + 1) * BC].reshape(BC * L, M)[leaf_idx, :]
        opc = opsA[c * BC:(c + 1) * BC]
        op0 = opc[db, di].astype(np.float32)[:, None]   # deal-ordered
        xac = np.ascontiguousarray((xc_flat * (1.0 - op0)).astype(f16).T)
        xoc = np.ascontiguousarray((xc_flat * op0).astype(f16).T)
        # bm rows (b-major child-res of levels 1..6): 0.5*op
        bm_rows = []
        for lvl in range(1, DEPTH):
            n = LEVEL_N[lvl]
            off = LEVEL_OFF[lvl]
            row = np.repeat(opc[:, off:off + n], 2, axis=1).reshape(1, -1)
            bm_rows.append(0.5 * row)
        bm = np.broadcast_to(np.concatenate(bm_rows, 1).astype(f16),
                             (128, MSK_TOT))
        in_maps.append({
            "xa": xac, "xo": xoc, "wa5": wa5, "wo5": wo5,
            "mb": np.ascontiguousarray(bm),
        })
    return in_maps


def postprocess(results):
    outs = []
    for c in range(N_CORES):
        r = np.asarray(results[c]["outT"]).reshape(128, 2, BC)
        outs.append(0.5 * np.transpose(r, (2, 1, 0)).reshape(BC, M))
    return np.concatenate(outs, axis=0).astype(np.float32)


def kernel(inputs, ops, W_and, W_or):
    nc = _get_nc()
    in_maps = make_in_maps(inputs, ops, W_and, W_or)
    res = bass_utils.run_bass_kernel_spmd(nc, in_maps, list(range(N_CORES)))
    return postprocess(res.results)
